# revision 24
# baseline (speedup 1.0000x reference)
"""Trainium2 Bass kernel for CSWin-style full attention with LePE.

Module (B=2, C=256, H=W=48, heads=8, head_dim=32):
    qkv = conv1x1(x)            -> q, k, v per head
    attn = softmax(k^T q * d^-0.5, over keys)
    out  = v @ attn + lepe(v)   (lepe = depthwise 3x3 conv + bias)
    out  = conv1x1(out) + b_proj

Sharding: 16 (batch, head) units over 8 cores -> each core owns one batch
index and two heads.  Each core computes its heads' attention + lepe and a
partial projection (w_proj columns of its channels); the host sums the 4
partials per batch and adds b_proj.

Per-core pipeline (matmul operands fp16, fp32 PSUM accumulation).  Every
matmul is zero-padded to a full 128x128 tile config (K and M padded with
zero rows/cols, all operands at partition base 0): mixed PE tiling
geometries interleaved with in-flight full-array matmuls produced
nondeterministic PSUM corruption on hardware -- padding is free anyway
since every matmul here is bound by its N-dim streaming, not K/M.

The schedule is built around ScalarE, whose exp stream (2 units x 2304^2
elements at 1 elem/lane/cycle) is the serial floor (~69us + per-call
overhead):

  - s = k^T q lands in [128, 2, 768] fp32 PSUM PAIR tiles (3 banks,
    double-buffered = 6 banks): ONE 1536-wide exp ACTIVATE per two
    k-tiles halves ScalarE's per-call overhead vs per-k-tile calls.
    Softmax max-subtraction is skipped (s*scale ~ N(0,1), exp cannot
    overflow); the scale rides the activation's free affine.  A dummy
    exp at t~0 prefetches the ACT table load off the critical path.
  - P*V for a window accumulates into two 1-bank chunks (512/256 wide)
    with a ones-column appended to v^T so the softmax denominator falls
    out of the same matmul.  Normalization is deferred to the [32, q]
    output: denominator row copied to partition 0 in SBUF (custom-DVE
    recip breaks with PSUM inputs / nonzero partition base), recip'd,
    broadcast with a stream shuffle, multiplied in.
  - lepe for BOTH heads in one 9-tap series of accumulating
    (shifted-)diag matmuls over a zero-padded [*, 50, 50] copy of v
    (SAME padding free from the zero border); b_lepe folded into the
    PSUM evacuation; out rows 0:64 = [lepeA; lepeB].
  - All big zero-fills run on the otherwise idle GpSimd engine, sliced
    so they never gate the qkv PSUM evacuations (keeps PE dense from
    t~4us, which keeps the HAM clock gate open).
  - Steady-state interleave: per pair-slot the PE runs 4 s-matmuls (one
    pair) + the previous window's PV matmuls for two k-tiles + a nibble
    of backlog work (v/v^T/lepe prep early, normalize/proj epilogues
    later), pacing the PE (~1.3us/slot) just under ScalarE's exp
    (~1.6us/slot).  The last window's PV is consumed same-window with a
    2-pair lag so the tail is just a couple of PV pairs + proj.
"""

import numpy as np

import concourse.bacc as bacc
import concourse.mybir as mybir
import concourse.tile as tile
from concourse.bass_utils import run_bass_kernel_spmd

F16 = mybir.dt.float16
F32 = mybir.dt.float32
ADD = mybir.AluOpType.add
EXP = mybir.ActivationFunctionType.Exp

B, C, H, W = 2, 256, 48, 48
N = H * W                      # 2304
HEADS, D = 8, 32
SCALE = D ** -0.5
NCORES = 8
KT = N // 128                  # 18 key tiles
NPAIR = KT // 2                # 9 s/exp pairs per window
QW = 768                       # q window width
NWIN = N // QW                 # 3 windows per unit
TAPS = [(dy, dx) for dy in (-1, 0, 1) for dx in (-1, 0, 1)]
# per-h 512-aligned PSUM chunk splits inside a [128, 2, 768] pair tile
SCHUNK = [[(0, 512), (512, 256)], [(0, 256), (256, 512)]]
PVCH = [(0, 384), (384, 384)]  # PV / proj / norm chunks of a 768 window
# each 384 chunk lives at offset 0 of its own 1-bank PSUM tile, so no
# matmul output crosses a bank boundary and no N=256 matmuls are needed


def _chunks(total, step):
    out, o = [], 0
    while o < total:
        out.append((o, min(step, total - o)))
        o += step
    return out


def _emit(nc, tc, pools, tensors):
    const, sb, pp, tmpp, rcpp, obp, ps_s, ps_w = pools
    x_d, wqk_d, wv_d, dg_d, bl_d, wp_d, out_d = tensors

    # ---- persistent SBUF tensors -----------------------------------
    x_sb = sb.tile([128, 2, N], F16, tag="x")
    # qk3 slabs: 0 = kA (lhsT u0), 1 = qB (rhs u1), 2 = kB (lhsT u1);
    # k slabs have rows 32:128 EXACT ZERO (they annihilate the rhs's
    # garbage rows); the qB slab's rows 32:128 only need to be finite
    # (DMA-filled from qk_tmp).  Unit 0's rhs is qk_tmp itself: qA is
    # already at rows 0:32 and rows 32:128 hold finite k/q data.
    qk3 = sb.tile([128, 3, N], F16, tag="qk3")
    qk_tmp = sb.tile([128, N], F16, tag="qktmp")
    vpad = sb.tile([128, 50, 50], F16, tag="vpad")      # rows 64:128 zero
    # vT2[:, u, kt, :]: [vT_u (32) | ones (1) | 95 don't-care cols]
    vT2 = sb.tile([128, 2, KT, 128], F16, tag="vT")
    lepe2 = sb.tile([64, N], F16, tag="lepe")           # rows: A 0:32, B 32:64
    # unit B's lepe re-based to partitions 0:32 (tensor ops need matching
    # partition ranges on every AP; SBUF->SBUF DMA does the re-basing)
    lepeB0 = sb.tile([32, N], F16, tag="lepeB0")
    y3 = sb.tile([128, N], F16, tag="y")   # A rows 0:32, B rows 64:96

    wqk = const.tile([128, 2, 128], F16, tag="wqk")
    wv = const.tile([128, 2, 128], F16, tag="wv")       # cols 64:128 zero
    dg2 = const.tile([128, 9, 128], F16, tag="dg")      # fused 2-head diag
    bl2 = const.tile([64, 1], F32, tag="bl")
    wp = const.tile([128, 2, 128], F16, tag="wp")
    dummy = const.tile([1, 1], F32, tag="dummy")
    # rc ping-pong: row 0 holds the per-chunk reciprocal; rows 1:32 stay
    # zero so the broadcast shuffle's unused source lanes are finite.
    rc = [sb.tile([32, 512], F32, tag=f"rc{i}", name=f"rc{i}")
          for i in range(2)]

    # ---- input DMAs, split over two queues (sync + gpsimd) so the
    # two channel halves of x land in parallel ------------------------
    for cc in range(2):
        nc.sync.dma_start(wqk[:, cc, :], wqk_d[cc])
    for cc in range(2):
        nc.sync.dma_start(x_sb[:, cc, 0:1152], x_d[cc, :, 0:1152])
    nc.sync.dma_start(wv[:, :, :], wv_d[:, :, :])
    nc.gpsimd.memset(dummy[:], 0.0)
    for cc in range(2):
        nc.gpsimd.dma_start(x_sb[:, cc, 1152:N], x_d[cc, :, 1152:N])
    nc.gpsimd.dma_start(dg2[:, :, :], dg_d[:, :, :])
    nc.gpsimd.dma_start(bl2[:, :], bl_d[:, :])
    nc.gpsimd.dma_start(wp[:, :, :], wp_d[:, :, :])

    # ---- zero fills ------------------------------------------------
    # memsets run at ~1 elem/lane/cycle on EVERY engine and partition
    # patterns with a nonzero base may span at most 32 partitions, so
    # zero only what matters: the two k slabs' rows 32:128.  Slab 0
    # (gates window 0) goes on the DVE, which is idle until the first
    # qkv evacuation; slab 2 (needed ~60us in) on GpSimd's tail.
    for p0 in (32, 64, 96):
        nc.vector.memset(qk3[p0:p0 + 32, 0, :], 0.0)
    # prefetch the exp table load (~2.7us) off the critical path
    scratch = rcpp.tile([1, 512], F32, tag="dn", name="scratch")
    nc.scalar.activation(scratch[0:1, 0:1], dummy[0:1, 0:1], EXP)
    # vpad: zero pad border for partitions 0:64 + full rows 64:128
    # (rows 64:128 multiply all-zero dg rows but must not be NaN)
    for p0 in (64, 96):
        nc.gpsimd.memset(vpad[p0:p0 + 32, :, :], 0.0)
    nc.gpsimd.memset(vpad[0:64, 0, :], 0.0)
    nc.gpsimd.memset(vpad[0:64, 49, :], 0.0)
    nc.gpsimd.memset(vpad[0:64, 1:49, 0], 0.0)
    nc.gpsimd.memset(vpad[0:64, 1:49, 49], 0.0)
    # vT2 ones column (cols 33:128 are don't-care: they only produce
    # PSUM rows 33:128, which are never read)
    for u in range(2):
        nc.gpsimd.memset(vT2[:, u, :, 32:33], 1.0)
    # y3 rows never written by norm; feed proj (NaN-safety)
    nc.gpsimd.memset(y3[32:64, :], 0.0)
    nc.gpsimd.memset(y3[96:128, :], 0.0)
    for rv in rc:
        nc.gpsimd.memset(rv[:], 0.0)
    for p0 in (32, 64, 96):
        nc.gpsimd.memset(qk3[p0:p0 + 32, 2, :], 0.0)

    # ---- qkv: q/k block [128, N] -> qk_tmp -> qk3 ------------------
    for c0, cw in _chunks(N, 512):
        t = ps_w.tile([128, 512], F32, tag="w", name="qk_ps")
        nc.tensor.matmul(t[:, :cw], wqk[:, 0, :], x_sb[:, 0, c0:c0 + cw],
                         start=True, stop=False)
        nc.tensor.matmul(t[:, :cw], wqk[:, 1, :], x_sb[:, 1, c0:c0 + cw],
                         start=False, stop=True)
        nc.scalar.copy(qk_tmp[:, c0:c0 + cw], t[:, :cw])
    # qk_tmp rows: qA 0:32 | kA 32:64 | qB 64:96 | kB 96:128
    nc.sync.dma_start(qk3[0:32, 0, 0:1024], qk_tmp[32:64, 0:1024])  # kA lo
    nc.sync.dma_start(qk3[0:32, 0, 1024:N], qk_tmp[32:64, 1024:N])  # kA hi
    nc.sync.dma_start(qk3[0:32, 1, :], qk_tmp[64:96, :])    # qB data
    nc.sync.dma_start(qk3[0:32, 2, :], qk_tmp[96:128, :])   # kB data
    # finite filler for qB's unused rows (annihilated by kB's zero rows)
    nc.gpsimd.dma_start(qk3[32:128, 1, :], qk_tmp[0:96, :])

    # ---- backlog of PE work consumed a nibble per pair-slot --------
    # items: (window_index, pair_lag_index_or_99, closure); early prep
    # uses window_index=-1 (always consumable)
    backlog = []

    def v_item(r0, nr):
        def run():
            c0, cw = r0 * W, nr * W
            t = ps_w.tile([128, 512], F32, tag="w", name="v_ps")
            nc.tensor.matmul(t[:, :cw], wv[:, 0, :], x_sb[:, 0, c0:c0 + cw],
                             start=True, stop=False)
            nc.tensor.matmul(t[:, :cw], wv[:, 1, :], x_sb[:, 1, c0:c0 + cw],
                             start=False, stop=True)
            nc.vector.tensor_copy(vpad[0:64, 1 + r0:1 + r0 + nr, 1:49],
                                  t[0:64, :cw])
        return run

    def vt_item(nts):
        def run():
            for nt in nts:
                t = ps_w.tile([128, 512], F32, tag="w", name="vt_ps")
                nc.tensor.matmul(t[:, 0:128],
                                 x_sb[:, 0, nt * 128:(nt + 1) * 128],
                                 wv[:, 0, :], start=True, stop=False)
                nc.tensor.matmul(t[:, 0:128],
                                 x_sb[:, 1, nt * 128:(nt + 1) * 128],
                                 wv[:, 1, :], start=False, stop=True)
                nc.vector.tensor_copy(vT2[:, 0, nt, 0:32], t[:, 0:32])
                nc.vector.tensor_copy(vT2[:, 1, nt, 0:32], t[:, 32:64])
        return run

    def lepe_items(chunks):
        # one pass: hold one PSUM tile per chunk across all 9 taps
        state = {}

        def tap_item(ti):
            def run():
                if ti == 0:
                    state["ts"] = [
                        ps_w.tile([128, 512], F32, tag="w",
                                  name=f"lp_{chunks[0][0]}_{i}")
                        for i, _ in enumerate(chunks)]
                dy, dx = TAPS[ti]
                for t, (r0, nr) in zip(state["ts"], chunks):
                    nc.tensor.matmul(
                        t[:, :nr * W], dg2[:, ti, :],
                        vpad[:, 1 + r0 + dy:1 + r0 + dy + nr, 1 + dx:49 + dx],
                        start=(ti == 0), stop=(ti == 8))
            return run

        def evac_item():
            def run():
                for t, (r0, nr) in zip(state["ts"], chunks):
                    c0, cw = r0 * W, nr * W
                    # per-unit evacs: every AP stays on its own partitions
                    nc.vector.tensor_scalar(
                        lepe2[0:32, c0:c0 + cw], t[0:32, :cw],
                        bl2[0:32, 0:1], None, ADD)
                    nc.vector.tensor_scalar(
                        lepe2[32:64, c0:c0 + cw], t[32:64, :cw],
                        bl2[32:64, 0:1], None, ADD)
                    # re-base unit B's slab to partitions 0:32 via DMA
                    nc.sync.dma_start(lepeB0[0:32, c0:c0 + cw],
                                      lepe2[32:64, c0:c0 + cw])
            return run

        return [tap_item(ti) for ti in range(9)] + [evac_item()]

    backlog += [v_item(r0, nr) for r0, nr in _chunks(H, 10)]
    backlog += [vt_item(list(range(n, min(n + 3, KT)))) for n in range(0, KT, 3)]
    lepe_marks = []
    for w, pair in enumerate([[(0, 10), (10, 10)], [(20, 10), (30, 10)],
                              [(40, 8)]]):
        lepe_marks += [(w, 0, it) for it in lepe_items(pair)]

    # ---- attention window machinery --------------------------------
    sic = [0]  # rc ping-pong counter

    def pv_pair(u, p_t, state, j, wi):
        """accumulate PV for k-tiles 2j, 2j+1 into pv chunk tiles.
        The chunk tiles are allocated lazily at j==0 so the ps_w ring
        order always matches backlog consumption order (eager alloc can
        invert ring deps against the PE FIFO -> runtime deadlock)."""
        def run():
            if j == 0:
                state["pv"] = [
                    ps_w.tile([128, 512], F32, tag="w", name=f"pvA{wi}"),
                    ps_w.tile([128, 512], F32, tag="w", name=f"pvB{wi}")]
            pv = state["pv"]
            for kt in (2 * j, 2 * j + 1):
                for ci, (c0, cw) in enumerate(PVCH):
                    nc.tensor.matmul(pv[ci][:, 0:cw], vT2[:, u, kt, :],
                                     p_t[:, kt, c0:c0 + cw],
                                     start=(kt == 0), stop=(kt == KT - 1))
        return run

    def norm_item(u, q0, state):
        def run():
            pv = state["pv"]
            for ci, (c0, cw) in enumerate(PVCH):
                rv = rc[sic[0] % 2]
                sic[0] += 1
                dn = rcpp.tile([1, 512], F32, tag="dn", name="dn")
                nc.vector.tensor_copy(dn[0:1, :cw], pv[ci][32:33, 0:cw])
                nc.vector.reciprocal_approx_fast(rv[0:1, :cw], dn[0:1, :cw])
                rbs = rcpp.tile([32, 512], F32, tag="rcp", name="rbs")
                nc.vector.stream_shuffle(rbs[0:32, :cw], rv[0:32, :cw],
                                         [0] * 32)
                tm = tmpp.tile([32, 512], F32, tag="tmp", name="tm")
                nc.vector.tensor_mul(tm[0:32, :cw], pv[ci][0:32, 0:cw],
                                     rbs[0:32, :cw])
                qs = q0 + c0
                if u == 0:
                    nc.vector.tensor_add(y3[0:32, qs:qs + cw],
                                         lepe2[0:32, qs:qs + cw],
                                         tm[0:32, :cw])
                else:
                    tmB = tmpp.tile([32, 512], F16, tag="tmB", name="tmB")
                    nc.vector.tensor_add(tmB[0:32, :cw],
                                         lepeB0[0:32, qs:qs + cw],
                                         tm[0:32, :cw])
                    nc.vector.tensor_copy(y3[64:96, qs:qs + cw],
                                          tmB[0:32, :cw])
        return run

    def proj_item(q0):
        def run():
            for mc in range(2):
                for c0, cw in PVCH:
                    po = ps_w.tile([128, 512], F32, tag="w", name="proj_ps")
                    nc.tensor.matmul(po[:, 0:cw], wp[:, mc, :],
                                     y3[:, q0 + c0:q0 + c0 + cw],
                                     start=True, stop=True)
                    ob = obp.tile([128, 512], F32, tag="ob", name="ob")
                    nc.vector.tensor_copy(ob[:, 0:cw], po[:, 0:cw])
                    nc.sync.dma_start(out_d[mc, :, q0 + c0:q0 + c0 + cw],
                                      ob[:, 0:cw])
        return run

    # run v + vT now: their matmuls follow the qkv series immediately
    # while the qk4 rearrange DMAs are still in flight (keeps PE dense
    # through the HAM warm-up window)
    n_now = 5 + 6
    for run in backlog[:n_now]:
        run()
    # lepe pass k is deadlined to window k: it produces exactly the lepe
    # columns window k's normalize consumes (at window k+1, slot 0)
    backlog = [(-1, 0, run) for run in backlog[n_now:]] + lepe_marks

    wins = [(u, w * QW) for u in range(2) for w in range(NWIN)]
    for wi, (u, q0) in enumerate(wins):
        p_t = pp.tile([128, KT, QW], F16, tag="p", name="p_t")
        state = {}
        for i in range(NPAIR):
            # s pair: k-tiles 2i, 2i+1 against this q window
            sp = ps_s.tile([128, 2, QW], F32, tag="s", name="sp")
            kslab = 0 if u == 0 else 2
            for h in range(2):
                kt = 2 * i + h
                for c0, cw in SCHUNK[h]:
                    qs = q0 + c0
                    rhs = (qk_tmp[:, qs:qs + cw] if u == 0
                           else qk3[:, 1, qs:qs + cw])
                    nc.tensor.matmul(
                        sp[:, h, c0:c0 + cw],
                        qk3[:, kslab, kt * 128:(kt + 1) * 128],
                        rhs, start=True, stop=True)
            nc.scalar.activation(p_t[:, 2 * i:2 * i + 2, :], sp[:, :, :],
                                 EXP, scale=SCALE)
            # this window's own PV pair becomes available next slot:
            # by the time the PE reaches it (behind slot i+1's s-pair),
            # exp(i) has finished -> no FIFO stall
            backlog.append((wi, i + 1, pv_pair(u, p_t, state, i, wi)))
            # consume what's ready: anything from earlier windows, own
            # pairs up to the lag, capped so a deep backlog (lepe during
            # window 0) cannot push the next s-pair past ScalarE's beat
            popped = 0
            while (backlog and popped < 3
                   and (backlog[0][0] < wi
                        or (backlog[0][0] == wi and backlog[0][1] <= i))):
                backlog.pop(0)[2]()
                popped += 1
        # epilogues: norm at the next window's slot 0; proj one slot
        # later (one ScalarE beat gives the DVE normalize time to finish
        # before proj's matmuls queue up behind it in the PE FIFO)
        backlog.append((wi + 1, 0, norm_item(u, q0, state)))
        if u == 1:
            backlog.append((wi + 1, 1, proj_item(q0)))
    # drain
    for _, _, run in backlog:
        run()


def _build():
    nc = bacc.Bacc("TRN2", target_bir_lowering=False, debug=False)

    x_d = nc.dram_tensor("x", [2, 128, N], F16, kind="ExternalInput")
    wqk_d = nc.dram_tensor("wqk", [2, 128, 128], F16, kind="ExternalInput")
    wv_d = nc.dram_tensor("wv", [128, 2, 128], F16, kind="ExternalInput")
    dg_d = nc.dram_tensor("dg", [128, 9, 128], F16, kind="ExternalInput")
    bl_d = nc.dram_tensor("bl", [64, 1], F32, kind="ExternalInput")
    wp_d = nc.dram_tensor("wp", [128, 2, 128], F16, kind="ExternalInput")
    out_d = nc.dram_tensor("out", [2, 128, N], F32, kind="ExternalOutput")

    with tile.TileContext(nc) as tc:
        with (
            tc.tile_pool(name="const", bufs=1) as const,
            tc.tile_pool(name="sb", bufs=1) as sb,
            tc.tile_pool(name="pp", bufs=3) as pp,
            tc.tile_pool(name="tmp", bufs=2) as tmpp,
            tc.tile_pool(name="rcp", bufs=2) as rcpp,
            tc.tile_pool(name="ob", bufs=4) as obp,
            tc.tile_pool(name="ps_s", bufs=2, space="PSUM") as ps_s,
            tc.tile_pool(name="ps_w", bufs=2, space="PSUM") as ps_w,
        ):
            _emit(nc, tc,
                  (const, sb, pp, tmpp, rcpp, obp, ps_s, ps_w),
                  (x_d, wqk_d, wv_d, dg_d, bl_d, wp_d, out_d))

    nc.compile()
    return nc


_NC = None


def _get_nc():
    global _NC
    if _NC is None:
        _NC = _build()
    return _NC


def _prep_core(c, x, w_qkv, w_lepe, b_lepe, w_proj):
    b = c // 4
    hA, hB = 2 * (c % 4), 2 * (c % 4) + 1
    xb = np.asarray(x[b], np.float32).reshape(C, N)
    w_qkv = np.asarray(w_qkv, np.float32)
    w_lepe = np.asarray(w_lepe, np.float32)
    b_lepe = np.asarray(b_lepe, np.float32)
    w_proj = np.asarray(w_proj, np.float32)

    rows = np.concatenate([
        w_qkv[96 * hA + 0:96 * hA + 32],       # qA
        w_qkv[96 * hA + 32:96 * hA + 64],      # kA
        w_qkv[96 * hB + 0:96 * hB + 32],       # qB
        w_qkv[96 * hB + 32:96 * hB + 64],      # kB
    ], axis=0)                                 # [128, 256]
    wqk = np.ascontiguousarray(rows.T.reshape(2, 128, 128)).astype(np.float16)

    # wv[c', cc, j]: v weights for both units, transposed; cols 64:128 zero
    wv = np.zeros((2, 128, 128), np.float32)
    wv[:, :, 0:32] = w_qkv[96 * hA + 64:96 * hA + 96].T.reshape(2, 128, 32)
    wv[:, :, 32:64] = w_qkv[96 * hB + 64:96 * hB + 96].T.reshape(2, 128, 32)
    wv = np.ascontiguousarray(wv.transpose(1, 0, 2)).astype(np.float16)

    # dg2[c', ti, c]: fused 2-head shifted diag; unit A channel c at vpad
    # row c -> out row c; unit B channel c at vpad row 32+c -> out row 32+c
    dg2 = np.zeros((128, 9, 128), np.float32)
    idx = np.arange(32)
    for ti, (dy, dx) in enumerate(TAPS):
        dg2[idx, ti, idx] = w_lepe[32 * hA:32 * hA + 32, 0, dy + 1, dx + 1]
        dg2[32 + idx, ti, 32 + idx] = w_lepe[32 * hB:32 * hB + 32, 0,
                                             dy + 1, dx + 1]
    dg2 = dg2.astype(np.float16)

    bl2 = np.zeros((64, 1), np.float32)
    bl2[0:32, 0] = b_lepe[32 * hA:32 * hA + 32]
    bl2[32:64, 0] = b_lepe[32 * hB:32 * hB + 32]

    # wp[c', mc, o']: proj weights; y rows A 0:32, B 64:96, rest zero
    wp = np.zeros((128, 2, 128), np.float32)
    wp[0:32] = w_proj[:, 32 * hA:32 * hA + 32].T.reshape(32, 2, 128)
    wp[64:96] = w_proj[:, 32 * hB:32 * hB + 32].T.reshape(32, 2, 128)
    wp = wp.astype(np.float16)

    return {
        "x": np.ascontiguousarray(xb.reshape(2, 128, N)).astype(np.float16),
        "wqk": wqk, "wv": wv, "dg": dg2, "bl": bl2, "wp": wp,
    }


_LAST_RES = None


def kernel(x, w_qkv, w_lepe, b_lepe, w_proj, b_proj, **_ignored):
    global _LAST_RES
    nc = _get_nc()
    in_maps = [_prep_core(c, x, w_qkv, w_lepe, b_lepe, w_proj)
               for c in range(NCORES)]
    res = run_bass_kernel_spmd(nc, in_maps, core_ids=list(range(NCORES)))
    _LAST_RES = res
    out = np.zeros((B, C, N), np.float32)
    for c in range(NCORES):
        out[c // 4] += res.results[c]["out"].reshape(C, N)
    out += np.asarray(b_proj, np.float32)[None, :, None]
    return out.reshape(B, C, H, W).astype(np.float32)
